# revision 13
# baseline (speedup 1.0000x reference)
import sys

sys.path.insert(0, "/opt/trn_rl_repo")

import numpy as np

import concourse.bass as bass  # noqa: F401
import concourse.tile as tile
from concourse import bacc, mybir
from concourse.bass_utils import run_bass_kernel_spmd

# Problem shapes (hardcoded per contract).
N, D = 16384, 2048
NCORES = 8
NSHARD = N // NCORES  # 2048 rows per core
HALF_LOG_2PI = 0.5 * float(np.log(2.0 * np.pi))

P = 128  # SBUF partitions
KC = D // P  # 16 k-chunks of 128 features
NBLK = 512  # n-columns per transposed block (= 1 PSUM bank of f32)
TB = NBLK // P  # 4 row-tiles per block
BLOCKS = NSHARD // NBLK  # 4 blocks per core

_PROGRAM = None
LAST_RESULT = None  # BassKernelResults of the most recent run (for test harness)


# V1 (previous session, ~54.7 us): fp16-convert + fp16 transpose matmuls +
# per-chunk reduce matmuls. PE streamed every element twice (transpose +
# reduce) = ~47 us busy, co-bottleneck with DMA.
V1_CFG = dict(
    slabs=(512, 1024, 512),
    pst_bufs=4,
    up_bufs=6,
    n_dve=4,  # balances DVE (fp16 convert + 16 chunks) against ACT (48 chunks)
    use_fp16=True,
    mm_transpose=True,
    two_phase=True,
    combine_dma=True,
    ring_pingpong=True,  # alternate SP/GPSIMD DMA queues: hides issue gaps
)

# V2 (~48.5 us steady-state = the ~48 us pure-DMA floor of this terminal's
# ~350 GB/s/core HBM stream; rel err 2.6e-4): no fp16 convert; PE
# transpose-mode on raw f32 viewed as float32r (1.5 cyc/row, BIR-verifier
# requires the whole operand chain declared f32r from DRAM); ACT
# Square(scale*pt+bias); DVE accumulates squared chunks elementwise (Pool
# engine adds measured 5+ us SLOWER - keep "vv"); one f32r reduce matmul
# per (block, accumulator) at 1 cyc/row (N=512>=256) instead of one per
# chunk -> 16x less PE reduce work. Single SP DMA queue: one queue already
# sustains the HBM cap; ping-pong/3-queue variants measured equal or worse.
DEFAULT_CFG = dict(arch="v2", acc_engines="vv")


def _build_v2(
    repeat=1,
    slabs=(512, 1024, 512),
    xb=4,
    pst_bufs=4,
    up_bufs=6,
    n_acc=2,
    acc_engines="vg",  # per-accumulator engine: v=DVE, g=GPSIMD(Pool)
    two_phase=True,
    stage=4,
    reduce_mode="f32r",  # f32r | f32 | bf16 (dtype of wacc + reduce matmul)
    load_pingpong=False,  # alternate x loads across SP/GPSIMD DMA queues
    store_eng="g",  # out-store DMA queue: g=GPSIMD, s=ACT
    pack_accs=False,  # 4 accs in 2 PSUM banks (partitions 0/32/64 + one) -> pst can use 6
    arch=None,  # ignored (dispatcher key)
):
    """out[n] = gamma - sum_k (p[k]*x[n,k] + q[k])**2.

    Per chunk (128 features x 512 rows): PE transpose-mode moves raw f32
    (viewed as float32r: 1.5 cyc/row, no fp16 convert) into PSUM; ACT does
    one Square(scale*pt+bias) pass; DVE/Pool accumulate squared chunks
    elementwise in f32 SBUF. One f32r reduce matmul per (block,
    accumulator) replaces the per-chunk reduce (16x less PE reduce work).
    """
    nc = bacc.Bacc(
        "TRN2",
        target_bir_lowering=False,
        debug=False,
        enable_asserts=False,
        num_devices=NCORES,
    )
    f32 = mybir.dt.float32
    f32r = mybir.dt.float32r

    x_ap = nc.dram_tensor("x", [NSHARD, D], f32r, kind="ExternalInput").ap()
    CW = 2 * KC + 1 + P
    cf_ap = nc.dram_tensor("coefs", [P, CW], f32, kind="ExternalInput").ap()
    idr_ap = nc.dram_tensor("identr", [P, P + 1], f32r, kind="ExternalInput").ap()
    out_ap = nc.dram_tensor("out", [1, NSHARD], f32, kind="ExternalOutput").ap()

    assert sum(slabs) == D and all(s % P == 0 for s in slabs)
    nslab = len(slabs)
    slab_off = [sum(slabs[:i]) for i in range(nslab)]
    chunk_slab = []
    for i, s in enumerate(slabs):
        for j in range(s // P):
            chunk_slab.append((i, j))

    with tile.TileContext(nc) as tc:
        with (
            tc.tile_pool(name="coef", bufs=1) as coef,
            tc.tile_pool(name="xp", bufs=xb) as xp,
            tc.tile_pool(name="up", bufs=up_bufs) as up,
            tc.tile_pool(name="wa", bufs=2) as wa,
            tc.tile_pool(name="obuf", bufs=2) as obuf,
            tc.tile_pool(name="pst", bufs=pst_bufs, space="PSUM") as pst,
            tc.tile_pool(name="psa", bufs=1 if pack_accs else 4, space="PSUM") as psa,
        ):
            cf_sb = coef.tile([P, CW], f32)
            nc.scalar.dma_start(cf_sb[:], cf_ap[:, :])
            p_sb = cf_sb[:, 0:KC]
            q_sb = cf_sb[:, KC : 2 * KC]
            g_sb = cf_sb[:, 2 * KC : 2 * KC + 1]
            idr_sb = coef.tile([P, P + 1], f32r)
            nc.scalar.dma_start(idr_sb[:], idr_ap[:, :])
            idr = idr_sb[:, 0:P]
            wdt = f32r if reduce_mode == "f32r" else f32
            if reduce_mode == "f32r":
                no_sb = idr_sb[:, P : P + 1]  # -1 column, f32r from DRAM
            else:
                no_t = coef.tile(
                    [P, 1], {"f32": f32, "bf16": mybir.dt.bfloat16}[reduce_mode]
                )
                nc.vector.memset(no_t[:], -1.0)
                no_sb = no_t[:]

            eng_map = {"v": nc.vector, "g": nc.gpsimd}
            sq = mybir.ActivationFunctionType.Square

            dma_seq = [0]

            def load_slab(b, s, xs):
                xt = xp.tile([P, TB * slabs[s]], f32r, tag=f"x{s}")
                r0 = b * TB * P
                src = x_ap[
                    r0 : r0 + TB * P, slab_off[s] : slab_off[s] + slabs[s]
                ].rearrange("(t p) c -> p t c", p=P)
                dst = xt[:].rearrange("p (t c) -> p t c", t=TB)
                eng = nc.sync
                if load_pingpong:
                    eng = [nc.sync, nc.gpsimd][dma_seq[0] % 2]
                    dma_seq[0] += 1
                eng.dma_start(dst, src)
                for t in range(TB):
                    xs[t][s] = xt[:, t * slabs[s] : (t + 1) * slabs[s]]

            def do_chunk(b, c, xs, waccs, first):
                if stage < 2:
                    return
                s, ck = chunk_slab[c]
                pt = pst.tile([P, NBLK], f32r)
                for t in range(TB):
                    nc.tensor.transpose(
                        pt[:, t * P : (t + 1) * P],
                        xs[t][s][:, ck * P : (ck + 1) * P],
                        idr,
                    )
                if stage < 3:
                    return
                ptf = pt[:].bitcast(f32)
                j = c % n_acc
                if stage < 4 or not first[j]:
                    w = up.tile([P, NBLK], wdt)
                    nc.scalar.activation(
                        w[:], ptf, sq, bias=q_sb[:, c : c + 1], scale=p_sb[:, c : c + 1]
                    )
                    if stage >= 4:
                        eng = eng_map[acc_engines[j % len(acc_engines)]]
                        eng.tensor_add(waccs[j][:], waccs[j][:], w[:])
                else:
                    # accumulator's first chunk: ACT writes it directly
                    nc.scalar.activation(
                        waccs[j][:],
                        ptf,
                        sq,
                        bias=q_sb[:, c : c + 1],
                        scale=p_sb[:, c : c + 1],
                    )
                    first[j] = False

            def finish_block(b, waccs, acc=None):
                ob = obuf.tile([1, NBLK], f32)
                if stage < 4:
                    nc.vector.memset(ob[:], 0.0)
                else:
                    if acc is None:
                        acc = psa.tile([1, NBLK], f32, name="acc")[:]
                    for j in range(n_acc):
                        if reduce_mode == "bf16":
                            wr = up.tile([P, NBLK], mybir.dt.bfloat16, tag="wb")
                            nc.vector.tensor_copy(wr[:], waccs[j][:])
                            wr = wr[:]
                        else:
                            wr = waccs[j][:]
                        nc.tensor.matmul(
                            acc, no_sb, wr, start=(j == 0), stop=(j == n_acc - 1)
                        )
                    nc.vector.tensor_scalar_add(ob[:], acc, g_sb[0:1, 0:1])
                oeng = nc.gpsimd if store_eng == "g" else nc.scalar
                oeng.dma_start(out_ap[0:1, b * NBLK : (b + 1) * NBLK], ob[:])

            def mk_waccs(b):
                if stage < 4:
                    return None
                return [
                    wa.tile([P, NBLK], wdt, tag=f"w{b}_{j}", name=f"wacc{b}_{j}")
                    for j in range(n_acc)
                ]

            for _rep in range(repeat):
                if not two_phase:
                    for b in range(BLOCKS):
                        xs = [[None] * nslab for _ in range(TB)]
                        for s in range(nslab):
                            load_slab(b, s, xs)
                        waccs = mk_waccs(b)
                        first = [True] * n_acc
                        for c in range(KC):
                            do_chunk(b, c, xs, waccs, first)
                        finish_block(b, waccs)
                else:
                    xs_all = [
                        [[None] * nslab for _ in range(TB)] for _ in range(BLOCKS)
                    ]
                    accs = [None] * BLOCKS
                    if pack_accs and stage >= 4:
                        # 3 accs share one bank at legal base partitions
                        # 0/32/64; the 4th gets its own bank (as V1 did).
                        acc3 = psa.tile([P, NBLK], f32, tag="acc3")
                        accl = psa.tile([1, NBLK], f32, tag="accl")
                        accs = [acc3[32 * b : 32 * b + 1, :] for b in range(3)] + [
                            accl[:]
                        ]
                    waccs_all = [mk_waccs(b) for b in range(BLOCKS)]
                    firsts = [[True] * n_acc for _ in range(BLOCKS)]
                    first_chunks = [
                        c for c in range(KC) if chunk_slab[c][0] < nslab - 1
                    ]
                    last_chunks = [
                        c for c in range(KC) if chunk_slab[c][0] == nslab - 1
                    ]
                    for b in range(BLOCKS):
                        for s in range(nslab - 1):
                            load_slab(b, s, xs_all[b])
                        for c in first_chunks:
                            do_chunk(b, c, xs_all[b], waccs_all[b], firsts[b])
                    s_last = nslab - 1
                    for b in range(BLOCKS):
                        load_slab(b, s_last, xs_all[b])
                        for c in last_chunks:
                            do_chunk(b, c, xs_all[b], waccs_all[b], firsts[b])
                        finish_block(b, waccs_all[b], accs[b])

    nc.compile()
    return nc


def _build_program(repeat=1, arch=None, **kw):
    if arch == "v2":
        return _build_v2(repeat=repeat, **kw)
    return _build_v1(repeat=repeat, **kw)


def _build_v1(
    repeat=1,
    xp_bufs=None,
    pst_bufs=4,
    up_bufs=6,
    slabs=(512, 1024, 512),
    n_dve=6,
    dma_only=False,
    two_phase=True,
    use_fp16=False,
    cvt_bufs=None,
    stage=4,
    mm_transpose=False,
    combine_dma=True,
    pack_accs=False,
    ring_pingpong=False,
):
    """stage: 0=dma only, 1=+convert, 2=+transpose, 3=+act/dve square,
    4=full (reduce matmul + gamma + store)."""
    if dma_only:
        stage = 0
    """out[n] = gamma - sum_k (p[k]*x[n,k] + q[k])**2, computed as:
    PE-transpose x into [k, n] layout, one fused ACT Square(p*x+q) pass
    (per-partition scale/bias = per-feature), then a [-1]*u matmul reduce
    over k accumulated in PSUM.

    repeat>1 re-runs the whole pass inside one NEFF (for differential
    HW timing only; results identical).
    """
    nc = bacc.Bacc(
        "TRN2",
        target_bir_lowering=False,
        debug=False,
        enable_asserts=False,
        num_devices=NCORES,
    )
    f32 = mybir.dt.float32
    bf16 = mybir.dt.bfloat16
    fp16 = mybir.dt.float16
    if mm_transpose:
        # transpose = regular matmul xh.T @ I (fp16, 1 cyc/row, no PE
        # transpose-mode toggling); regular matmuls must write fp32 PSUM
        use_fp16 = True
    tdt = f32 if (mm_transpose or not use_fp16) else fp16  # PSUM dtype

    x_ap = nc.dram_tensor("x", [NSHARD, D], f32, kind="ExternalInput").ap()
    # One packed coef tensor: cols [0,KC)=p, [KC,2KC)=q, [2KC]=gamma(row 0),
    # [2KC+1, 2KC+1+P)=identity. Single DMA instead of five.
    CW = 2 * KC + 1 + P
    cf_ap = nc.dram_tensor("coefs", [P, CW], f32, kind="ExternalInput").ap()
    idh_ap = None
    if use_fp16:
        idh_ap = nc.dram_tensor("identh", [P, P], fp16, kind="ExternalInput").ap()
    out_ap = nc.dram_tensor("out", [1, NSHARD], f32, kind="ExternalOutput").ap()

    assert sum(slabs) == D and all(s % P == 0 for s in slabs)
    nslab = len(slabs)
    slab_off = [sum(slabs[:i]) for i in range(nslab)]  # k-col start of slab i
    chunk_slab = []  # chunk index -> (slab idx, chunk-within-slab)
    for i, s in enumerate(slabs):
        for j in range(s // P):
            chunk_slab.append((i, j))
    if xp_bufs is None:
        # with the fp16 convert stage, raw f32 tiles are transient
        xp_bufs = 2 if use_fp16 else BLOCKS
    if cvt_bufs is None:
        cvt_bufs = BLOCKS  # converted tiles: whole shard resident

    with tile.TileContext(nc) as tc:
        with (
            tc.tile_pool(name="coef", bufs=1) as coef,
            tc.tile_pool(name="xp", bufs=xp_bufs) as xp,
            tc.tile_pool(name="xc", bufs=cvt_bufs) as xc,
            tc.tile_pool(name="up", bufs=up_bufs) as up,
            tc.tile_pool(name="obuf", bufs=2) as obuf,
            tc.tile_pool(name="pst", bufs=pst_bufs, space="PSUM") as pst,
            tc.tile_pool(
                name="psa",
                bufs=1 if pack_accs else (BLOCKS if two_phase else 2),
                space="PSUM",
            ) as psa,
        ):
            # Constant load goes on ACT's DMA ring so the SP ring is free to
            # start streaming x immediately (the two rings are independent).
            cf_sb = coef.tile([P, CW], f32)
            nc.scalar.dma_start(cf_sb[:], cf_ap[:, :])
            p_sb = cf_sb[:, 0:KC]
            q_sb = cf_sb[:, KC : 2 * KC]
            g_sb = cf_sb[:, 2 * KC : 2 * KC + 1]
            id_sb = cf_sb[:, 2 * KC + 1 : 2 * KC + 1 + P]
            no_sb = coef.tile([P, 1], bf16)
            nc.vector.memset(no_sb[:], -1.0)
            if use_fp16:
                idh_sb = coef.tile([P, P], fp16)
                nc.scalar.dma_start(idh_sb[:], idh_ap[:, :])
                id_t = idh_sb
            else:
                id_t = id_sb

            # DVE takes n_dve of the KC chunks per block, preferring the
            # last slab's chunks (they're on the post-DMA critical tail).
            dve_order = [9, 11, 13, 15, 1, 3, 5, 7, 8, 10, 12, 14, 0, 2, 4, 6]
            dve_set = set(dve_order[:n_dve])

            def do_chunk(b, c, xs, acc):
                if stage < 2:
                    return
                s, ck = chunk_slab[c]
                pt = pst.tile([P, NBLK], tdt)
                for t in range(TB):
                    if mm_transpose:
                        nc.tensor.matmul(
                            pt[:, t * P : (t + 1) * P],
                            xs[t][s][:, ck * P : (ck + 1) * P],
                            id_t,
                            start=True,
                            stop=True,
                        )
                    else:
                        nc.tensor.transpose(
                            pt[:, t * P : (t + 1) * P],
                            xs[t][s][:, ck * P : (ck + 1) * P],
                            id_t,
                        )
                if stage < 3:
                    return
                if c in dve_set:
                    # fp16 u: |u| <= ~5e4 even for worst-case deg/std, and
                    # fp16's 10-bit mantissa quarters the rounding noise of
                    # bf16 before the square.
                    u = up.tile([P, NBLK], fp16)
                    nc.vector.tensor_scalar(
                        u[:],
                        pt[:],
                        p_sb[:, c : c + 1],
                        q_sb[:, c : c + 1],
                        mybir.AluOpType.mult,
                        mybir.AluOpType.add,
                    )
                    w = up.tile([P, NBLK], bf16)
                    nc.vector.tensor_mul(w[:], u[:], u[:])
                else:
                    w = up.tile([P, NBLK], bf16)
                    nc.scalar.activation(
                        w[:],
                        pt[:],
                        mybir.ActivationFunctionType.Square,
                        bias=q_sb[:, c : c + 1],
                        scale=p_sb[:, c : c + 1],
                    )
                if stage < 4:
                    return
                nc.tensor.matmul(
                    acc[:],
                    no_sb[:],
                    w[:],
                    start=(c == 0),
                    stop=(c == KC - 1),
                )

            dma_seq = [0]

            def load_slab(b, s, xs):
                # One DMA per (block, slab) carries all TB row-tiles: the
                # DRAM side is read as [(t p) c -> p (t c)], so partition p
                # receives rows r0+t*128+p for t=0..TB-1. Fewer, bigger DMAs
                # (~0.25us fixed cost each on the serial ring).
                eng = nc.sync
                if ring_pingpong:
                    # rotate across DMA queues (SP HWDGE, GPSIMD SWDGE, and
                    # optionally ACT HWDGE) so consecutive transfers hide
                    # each other's issue/completion gap
                    nq = int(ring_pingpong) + 1
                    eng = [nc.sync, nc.gpsimd, nc.scalar][dma_seq[0] % nq]
                    dma_seq[0] += 1
                xt = xp.tile([P, TB * slabs[s]], f32, tag=f"x{s}")
                r0 = b * TB * P
                if combine_dma:
                    src = x_ap[
                        r0 : r0 + TB * P, slab_off[s] : slab_off[s] + slabs[s]
                    ].rearrange("(t p) c -> p t c", p=P)
                    dst = xt[:].rearrange("p (t c) -> p t c", t=TB)
                    eng.dma_start(dst, src)
                else:
                    for t in range(TB):
                        eng.dma_start(
                            xt[:, t * slabs[s] : (t + 1) * slabs[s]],
                            x_ap[
                                r0 + t * P : r0 + (t + 1) * P,
                                slab_off[s] : slab_off[s] + slabs[s],
                            ],
                        )
                if use_fp16 and stage >= 1:
                    # fp16 copy on DVE (2x single-src mode) so the PE
                    # transpose runs at 1 cyc/row instead of fp32's 2.
                    xh = xc.tile([P, TB * slabs[s]], fp16, tag=f"xc{s}")
                    nc.vector.tensor_copy(xh[:], xt[:])
                    big = xh
                else:
                    big = xt
                for t in range(TB):
                    xs[t][s] = big[:, t * slabs[s] : (t + 1) * slabs[s]]

            def finish_block(b, acc):
                ob = obuf.tile([1, NBLK], f32)
                if stage < 4:
                    nc.vector.memset(ob[:], 0.0)
                else:
                    nc.vector.tensor_scalar_add(ob[:], acc[:], g_sb[0:1, 0:1])
                # out stores on GPSIMD's queue (they depend on compute; on
                # the SP ring they would stall later x loads).
                oeng = nc.gpsimd if store_eng == "g" else nc.scalar
                oeng.dma_start(out_ap[0:1, b * NBLK : (b + 1) * NBLK], ob[:])

            if not two_phase:
                for _rep in range(repeat):
                    for b in range(BLOCKS):
                        xs = [[None] * nslab for _ in range(TB)]
                        for s in range(nslab):
                            load_slab(b, s, xs)
                        acc = psa.tile([1, NBLK], f32)
                        for c in range(KC):
                            do_chunk(b, c, xs, acc)
                        finish_block(b, acc)

              # (two-phase below)
            else:
              for _rep in range(repeat):
                # Two-phase schedule: every block's slab-0 work first (all
                # BLOCKS accumulators stay live in PSUM), then slab-1 per
                # block. The post-last-DMA tail is one block's last-slab
                # chunks only.
                xs_all = [[[None] * nslab for _ in range(TB)] for _ in range(BLOCKS)]
                accs = [None] * BLOCKS
                if pack_accs:
                    # three block accumulators share one PSUM bank at
                    # partitions 0/32/64 (the legal AP base partitions);
                    # the fourth gets its own bank. Frees 2 banks for pst.
                    acc3 = psa.tile([128, NBLK], f32, tag="acc3")
                    accl = psa.tile([1, NBLK], f32, tag="accl")
                    accs = [acc3[32 * b : 32 * b + 1, :] for b in range(3)] + [accl]
                first_chunks = [c for c in range(KC) if chunk_slab[c][0] < nslab - 1]
                last_chunks = [c for c in range(KC) if chunk_slab[c][0] == nslab - 1]
                for b in range(BLOCKS):
                    for s in range(nslab - 1):
                        load_slab(b, s, xs_all[b])
                    if not pack_accs:
                        acc = psa.tile([1, NBLK], f32)
                        accs[b] = acc
                    for c in first_chunks:
                        do_chunk(b, c, xs_all[b], accs[b])
                s_last = nslab - 1
                for b in range(BLOCKS):
                    load_slab(b, s_last, xs_all[b])
                    for c in last_chunks:
                        do_chunk(b, c, xs_all[b], accs[b])
                    ob = obuf.tile([1, NBLK], f32)
                    if stage < 4:
                        nc.vector.memset(ob[:], 0.0)
                    else:
                        nc.vector.tensor_scalar_add(ob[:], accs[b][:], g_sb[0:1, 0:1])
                    # out stores on GPSIMD's queue (they depend on compute; on
                    # the SP ring they would stall later x loads).
                    nc.gpsimd.dma_start(
                        out_ap[0:1, b * NBLK : (b + 1) * NBLK], ob[:]
                    )

    nc.compile()
    return nc


def kernel(x, raw_params, edges, _trace=False):
    global _PROGRAM, LAST_RESULT
    x = np.ascontiguousarray(np.asarray(x, dtype=np.float32))
    raw_params = np.asarray(raw_params, dtype=np.float64)
    edges = np.asarray(edges)
    assert x.shape == (N, D), x.shape

    # Tiny host-side coefficient math (O(D); the O(N*D) pass runs on device).
    means = np.tanh(raw_params[:D]) * 2.0
    stds = np.logaddexp(0.0, raw_params[D:]) + 1e-6  # softplus + eps
    deg = np.zeros(D, dtype=np.float64)
    np.add.at(deg, edges.reshape(-1), 1.0)
    p = np.sqrt(0.5 * deg) / stds
    q = -means * p
    gamma = float(-np.sum(deg * (np.log(stds) + HALF_LOG_2PI)))

    p2d = p.reshape(KC, P).T.astype(np.float32)
    q2d = q.reshape(KC, P).T.astype(np.float32)
    coefs = np.zeros((P, 2 * KC + 1 + P), dtype=np.float32)
    coefs[:, 0:KC] = p2d
    coefs[:, KC : 2 * KC] = q2d
    coefs[:, 2 * KC] = gamma
    coefs[:, 2 * KC + 1 :] = np.eye(P, dtype=np.float32)

    if _PROGRAM is None:
        _PROGRAM = _build_program(**DEFAULT_CFG)
    nc = _PROGRAM

    in_maps = []
    for c in range(NCORES):
        shard = x[c * NSHARD : (c + 1) * NSHARD]
        m = {"x": shard, "coefs": coefs}
        if DEFAULT_CFG.get("arch") == "v2":
            idn = np.concatenate(
                [np.eye(P, dtype=np.float32), np.full((P, 1), -1.0, np.float32)], axis=1
            )
            m["identr"] = idn
        elif DEFAULT_CFG.get("use_fp16"):
            m["identh"] = np.eye(P, dtype=np.float16)
        in_maps.append(m)

    LAST_RESULT = run_bass_kernel_spmd(
        nc, in_maps, core_ids=list(range(NCORES)), trace=_trace
    )
    out = np.concatenate(
        [LAST_RESULT.results[c]["out"].reshape(-1) for c in range(NCORES)]
    )
    return out.astype(np.float32)

